# revision 1
# baseline (speedup 1.0000x reference)
"""Trainium2 Bass kernel for nn_FastSelfAttention (sparse_attention).

Math (per batch b, x = hidden_states[b], all biases folded):
    mq = x@Wq.T + bq ; q_w = softmax_S((mq@Wqa.T + bqa)*s)
    pooled_q = einsum(q_w, mq) ; mqk = (x@Wk.T + bk) * pooled_q
    k_w = softmax_S((mqk@Wka.T + bka)*s) ; pooled_k = einsum(k_w, mqk)
    out = (pooled_k * mq)@Wt.T + bt + mq

Algebraic collapse used here (validated to ~6e-7 rel vs reference):
    q_score = x@A1.T + c1,          A1 = s*Wqa@Wq (host)
    xq_pool = softmax-pool of x itself (unnormalized exp + denom matmul)
    pooled_q[hd] = xq_pool[head(hd)].Wq[hd] + bq[hd]
    A2.T = Wk.T @ ((s*K2*Wka).T * pooled_q)    (device, tiny)
    k path symmetric; pooled_k = pooled_q * (xk_pool[h].Wk[hd] + bk[hd])
    M1 = KAPPA*pooled_k[:,None]*Wt.T ; W_final = Wq.T@M1/KAPPA + Wq.T
    out = x @ W_final + (bq@M1/KAPPA... + bt)      <- ONE big matmul

Sharding: data-parallel over batch, one batch row per NeuronCore (8 cores).
All x-side matmuls run in fp16 (1 cyc/row on PE, 2-byte for DMA-xbar
transposes); accumulation is fp32 in PSUM. End-to-end numeric error vs the
fp32 reference is ~3e-4 relative-to-scale.
"""

import numpy as np

_B, _S, _H, _NH = 8, 4096, 512, 8
_D = _H // _NH
_SCALE = 1.0 / float(np.sqrt(_D))
_K2 = 64.0        # extra scaling on A2 path so fp16 entries stay normal
_KAPPA = 1024.0   # scaling on the M1/W_final correction path

_NT = _S // 128   # 32 sequence tiles
_KT = _H // 128   # 4 feature tiles
_NCH = _S // 512  # 8 score chunks

_BUILT = {}
LAST_RESULTS = None


def _build(with_bias_final):
    import concourse.bacc as bacc
    import concourse.tile as tile
    from concourse import mybir
    from contextlib import ExitStack

    f32 = mybir.dt.float32
    f16 = mybir.dt.float16
    Exp = mybir.ActivationFunctionType.Exp

    nc = bacc.Bacc(
        "TRN2",
        target_bir_lowering=False,
        debug=False,
        enable_asserts=False,
        num_devices=8,
    )

    def din(name, shape, dt=f32):
        return nc.dram_tensor(name, shape, dt, kind="ExternalInput").ap()

    x_d = din("x", [_S, _H])
    a1t_d = din("a1t", [_H, _NH], f16)        # (s*Wqa@Wq).T
    wkast_d = din("wkast", [_H, _NH], f32)    # (s*K2*Wka).T
    wqT16_d = din("wqT16", [_H, _H], f16)     # Wq.T
    wqT32_d = din("wqT32", [_H, _H], f32)     # Wq.T fp32 (W_final add)
    wqn16_d = din("wqn16", [_H, _H], f16)     # Wq natural (lhsT of Wq.T@M1)
    wkn16_d = din("wkn16", [_H, _H], f16)     # Wk natural (lhsT of A2.T)
    wkT16_d = din("wkT16", [_H, _H], f16)     # Wk.T
    wtTk16_d = din("wtTk16", [_H, _H], f16)   # KAPPA*Wt.T
    ident_d = din("ident", [128, 128], f16)
    c1_d = din("c1", [_NH, 1], f32)           # s*(Wqa@bq+bqa): q exp bias
    sbka_d = din("sbka", [_NH, 1], f32)       # s*bka
    bqhd_d = din("bqhd", [_H, 1], f32)
    bkhd16_d = din("bkhd16", [_H, 1], f16)
    bkhd32_d = din("bkhd32", [_H, 1], f32)
    if with_bias_final:
        bq16_d = din("bq16", [_H, 1], f16)
        bt_d = din("bt", [1, _H], f32)
    out_d = nc.dram_tensor("out", [_S, _H], f32, kind="ExternalOutput").ap()

    with tile.TileContext(nc) as tc, ExitStack() as ctx:
        wpool = ctx.enter_context(tc.tile_pool(name="wpool", bufs=1))
        xpool = ctx.enter_context(tc.tile_pool(name="xpool", bufs=1))
        spool = ctx.enter_context(tc.tile_pool(name="spool", bufs=1))
        opool = ctx.enter_context(tc.tile_pool(name="opool", bufs=4))
        dpool = ctx.enter_context(tc.tile_pool(name="dpool", bufs=1, space="DRAM"))
        pscore = ctx.enter_context(tc.tile_pool(name="pscore", bufs=2, space="PSUM"))
        pacc = ctx.enter_context(tc.tile_pool(name="pacc", bufs=1, space="PSUM"))
        psmall = ctx.enter_context(tc.tile_pool(name="psmall", bufs=2, space="PSUM"))
        pbig = ctx.enter_context(tc.tile_pool(name="pbig", bufs=3, space="PSUM"))

        def load_w(src, name):
            """[H, C] dram -> [128, H//128, C] sbuf (feature tiles on partitions)."""
            t = wpool.tile([128, src.shape[0] // 128, src.shape[1]], src.dtype, name=name)
            nc.sync.dma_start(t[:], src.rearrange("(t p) c -> p t c", p=128))
            return t

        a1t = load_w(a1t_d, "a1t")
        wkast = load_w(wkast_d, "wkast")
        wqT16 = load_w(wqT16_d, "wqT16")
        wqT32 = load_w(wqT32_d, "wqT32")
        wqn16 = load_w(wqn16_d, "wqn16")
        wkn16 = load_w(wkn16_d, "wkn16")
        wkT16 = load_w(wkT16_d, "wkT16")
        wtTk16 = load_w(wtTk16_d, "wtTk16")
        bqhd = load_w(bqhd_d, "bqhd")
        bkhd16 = load_w(bkhd16_d, "bkhd16")
        bkhd32 = load_w(bkhd32_d, "bkhd32")
        ident = wpool.tile([128, 128], f16, name="ident")
        nc.sync.dma_start(ident[:], ident_d[:])
        c1 = wpool.tile([_NH, 1], f32, name="c1")
        nc.sync.dma_start(c1[:], c1_d[:])
        sbka = wpool.tile([_NH, 1], f32, name="sbka")
        nc.sync.dma_start(sbka[:], sbka_d[:])

        # ---- x preprocessing: fp32 -> fp16 (SWDGE cast to DRAM), then natural
        # and xbar-transposed loads.
        # x_nat uses a p-major sequence layout: x_nat[p, t, :] = x[p*32+t, :].
        # This matches the flattening the 3D-output xbar transpose produces
        # for exp_nat, so the pooling contraction enumerates s consistently.
        x16_d = dpool.tile([_S, _H], f16, name="x16_d")
        CH = _S // 4
        for c in range(4):
            nc.gpsimd.dma_start(
                x16_d[c * CH:(c + 1) * CH, :], x_d[c * CH:(c + 1) * CH, :]
            )
        x_nat = xpool.tile([128, _NT, _H], f16, name="x_nat")
        x16_pm = x16_d.rearrange("(c p t) i -> c p t i", c=4, p=128)
        for c in range(4):
            nc.sync.dma_start(x_nat[:, 8 * c:8 * (c + 1), :], x16_pm[c])
        xT = xpool.tile([128, _KT, _S], f16, name="xT")
        for it in range(_KT):
            for c in range(4):
                nc.sync.dma_start(
                    xT[:, it, c * CH:(c + 1) * CH],
                    x16_d[c * CH:(c + 1) * CH, it * 128:(it + 1) * 128],
                    transpose=True,
                )

        def softmax_pool(score_lhsT, exp_scale, bias_ap, pfx):
            """scores (x-contraction) -> exp -> transpose -> pool of x.

            Returns (pool_f16 [8,512], poolT_f16 [128,KT,8])."""
            exp_sb = spool.tile([16, _S], f16, name=f"{pfx}_exp_sb", tag=f"{pfx}_exp_sb")
            nc.gpsimd.memset(exp_sb[:], 0.0)
            denp = spool.tile([_NH, _NCH], f32, name=f"{pfx}_denp", tag=f"{pfx}_denp")
            for ch in range(_NCH):
                ps = pscore.tile([_NH, 512], f32, name=f"{pfx}_ps", tag="score_ps")
                for kt in range(_KT):
                    nc.tensor.matmul(
                        ps[:],
                        score_lhsT[:, kt, :],
                        xT[:, kt, ch * 512:(ch + 1) * 512],
                        start=(kt == 0),
                        stop=(kt == _KT - 1),
                    )
                nc.scalar.activation(
                    exp_sb[0:_NH, ch * 512:(ch + 1) * 512],
                    ps[:],
                    Exp,
                    bias=bias_ap,
                    scale=exp_scale,
                    accum_out=denp[:, ch:ch + 1],
                )
            exp_d = dpool.tile([16, _S], f16, name=f"{pfx}_exp_d", tag=f"{pfx}_exp_d")
            exp_nat = xpool.tile([128, _NT, 16], f16, name=f"{pfx}_exp_nat",
                                 tag=f"{pfx}_exp_nat")
            CHE = _S // 4
            for c in range(4):
                nc.gpsimd.dma_start(exp_d[:, c * CHE:(c + 1) * CHE],
                                     exp_sb[:, c * CHE:(c + 1) * CHE])
                nc.sync.dma_start(exp_nat[:, 8 * c:8 * (c + 1), :],
                                  exp_d[:, c * CHE:(c + 1) * CHE], transpose=True)
            acc = pacc.tile([_NH, 512], f32, name=f"{pfx}_acc", tag="pool_acc")
            for t in range(_NT):
                nc.tensor.matmul(acc[:], exp_nat[:, t, 0:_NH], x_nat[:, t, :],
                                 start=(t == 0), stop=(t == _NT - 1))
            d4 = spool.tile([_NH, 4], f32, name=f"{pfx}_d4", tag=f"{pfx}_d4")
            nc.vector.tensor_add(d4[:], denp[:, 0:4], denp[:, 4:8])
            d2 = spool.tile([_NH, 2], f32, name=f"{pfx}_d2", tag=f"{pfx}_d2")
            nc.vector.tensor_add(d2[:], d4[:, 0:2], d4[:, 2:4])
            denf = spool.tile([_NH, 1], f32, name=f"{pfx}_denf", tag=f"{pfx}_denf")
            nc.vector.tensor_add(denf[:], d2[:, 0:1], d2[:, 1:2])
            rec = spool.tile([_NH, 1], f32, name=f"{pfx}_rec", tag=f"{pfx}_rec")
            nc.vector.reciprocal(rec[:], denf[:])
            pool = spool.tile([_NH, 512], f16, name=f"{pfx}_pool", tag=f"{pfx}_pool")
            nc.vector.tensor_scalar_mul(pool[:], acc[:], rec[:])
            poolT = spool.tile([128, _KT, _NH], f16, name=f"{pfx}_poolT",
                               tag=f"{pfx}_poolT")
            for blk in range(_KT):
                pt = psmall.tile([128, _NH], f16, name=f"{pfx}_pt", tag="small_ps")
                nc.tensor.transpose(
                    pt[:], pool[0:_NH, blk * 128:(blk + 1) * 128], ident[0:_NH, 0:_NH]
                )
                nc.vector.tensor_copy(poolT[:, blk, :], pt[:])
            return pool, poolT

        def pooled_vec(wT16, poolT, badd, name):
            """pooled[hd] = pool[head(hd)] . W[hd,:] + b[hd]  -> [128, KT, 1] f32."""
            pv = spool.tile([128, _KT, 1], f32, name=name, tag=name)
            for it in range(_KT):
                pm = psmall.tile([128, _NH], f32, name=f"{name}_pm", tag="small_ps")
                for kt in range(_KT):
                    nc.tensor.matmul(
                        pm[:],
                        wT16[:, kt, it * 128:(it + 1) * 128],
                        poolT[:, kt, :],
                        start=(kt == 0),
                        stop=(kt == _KT - 1),
                    )
                for half in range(2):
                    sl = slice(64 * half, 64 * (half + 1))
                    col = 2 * it + half
                    nc.vector.tensor_add(
                        pv[sl, it, :], pm[sl, col:col + 1], badd[sl, it, :]
                    )
            return pv

        # ---- q path
        _, poolqT = softmax_pool(a1t, 1.0, c1[0:_NH, :], "q")
        pq = pooled_vec(wqT16, poolqT, bqhd, "pq")

        # ---- A2 = Wk.T @ (wkast * pq)  (already transposed), c2 bias
        wkapq = spool.tile([128, _KT, _NH], f16, name="wkapq")
        for ht in range(_KT):
            nc.vector.tensor_scalar_mul(wkapq[:, ht, :], wkast[:, ht, :], pq[:, ht, :])
        a2T = spool.tile([128, _KT, _NH], f16, name="a2T")
        for it in range(_KT):
            pa = psmall.tile([128, _NH], f32, name="a2_pa", tag="small_ps")
            for ht in range(_KT):
                nc.tensor.matmul(
                    pa[:],
                    wkn16[:, ht, it * 128:(it + 1) * 128],
                    wkapq[:, ht, :],
                    start=(ht == 0),
                    stop=(ht == _KT - 1),
                )
            nc.vector.tensor_copy(a2T[:, it, :], pa[:])
        pc2 = psmall.tile([_NH, 1], f32, name="pc2", tag="small_ps")
        for ht in range(_KT):
            nc.tensor.matmul(pc2[:], wkapq[:, ht, :], bkhd16[:, ht, :],
                             start=(ht == 0), stop=(ht == _KT - 1))
        c2b = spool.tile([_NH, 1], f32, name="c2b")
        nc.vector.tensor_scalar(
            c2b[:], pc2[:], 1.0 / _K2, sbka[0:_NH, :],
            __import__("concourse.mybir", fromlist=["AluOpType"]).AluOpType.mult,
            __import__("concourse.mybir", fromlist=["AluOpType"]).AluOpType.add,
        )

        # ---- k path
        _, poolkT = softmax_pool(a2T, 1.0 / _K2, c2b[0:_NH, :], "k")
        prek = pooled_vec(wkT16, poolkT, bkhd32, "prek")
        pk = spool.tile([128, _KT, 1], f32, name="pk")
        for it in range(_KT):
            nc.vector.tensor_mul(pk[:, it, :], prek[:, it, :], pq[:, it, :])

        # ---- W_final = Wq.T @ (KAPPA*pk*Wt.T) / KAPPA + Wq.T
        m1 = spool.tile([128, _KT, _H], f16, name="m1")
        for jt in range(_KT):
            nc.vector.tensor_scalar_mul(m1[:, jt, :], wtTk16[:, jt, :], pk[:, jt, :])
        wf16 = spool.tile([128, _KT, _H], f16, name="wf16")
        for it in range(_KT):
            pw = pbig.tile([128, _H], f32, name="pw", tag="big_ps")
            for jt in range(_KT):
                nc.tensor.matmul(
                    pw[:],
                    wqn16[:, jt, it * 128:(it + 1) * 128],
                    m1[:, jt, :],
                    start=(jt == 0),
                    stop=(jt == _KT - 1),
                )
            wtmp = opool.tile([128, _H], f32, name="wtmp", tag="wtmp", bufs=2)
            nc.scalar.mul(wtmp[:], pw[:], 1.0 / _KAPPA)
            nc.vector.tensor_add(wf16[:, it, :], wtmp[:], wqT32[:, it, :])

        if with_bias_final:
            bq16 = load_w(bq16_d, "bq16")
            bt_sb = wpool.tile([1, _H], f32, name="bt_sb")
            nc.sync.dma_start(bt_sb[:], bt_d[:])
            pbf = psmall.tile([1, _H], f32, name="pbf", tag="small_ps")
            for jt in range(_KT):
                nc.tensor.matmul(pbf[:], bq16[:, jt, :], m1[:, jt, :],
                                 start=(jt == 0), stop=(jt == _KT - 1))
            bft = spool.tile([1, _H], f32, name="bft")
            nc.scalar.mul(bft[:], pbf[:], 1.0 / _KAPPA)
            bf16 = spool.tile([1, _H], f16, name="bf16")
            nc.vector.tensor_add(bf16[:], bft[:], bt_sb[:])
            one_row = spool.tile([1, 128], f16, name="one_row")
            nc.vector.memset(one_row[:], 1.0)

        # ---- final: out = x @ W_final (+ b_final)
        out_pm = out_d.rearrange("(t p) m -> p t m", p=128)
        GRP = 4
        for st in range(_NT):
            if st % GRP == 0:
                ot = opool.tile([128, GRP, _H], f32, name="ot", tag="ot", bufs=2)
            pf = pbig.tile([128, _H], f32, name="pf", tag="big_ps")
            for it in range(_KT):
                nc.tensor.matmul(
                    pf[:],
                    xT[:, it, st * 128:(st + 1) * 128],
                    wf16[:, it, :],
                    start=(it == 0),
                    stop=(it == _KT - 1 and not with_bias_final),
                )
            if with_bias_final:
                nc.tensor.matmul(pf[:], one_row[:], bf16[:], start=False, stop=True)
            if st % 2 == 0:
                nc.scalar.copy(ot[:, st % GRP, :], pf[:])
            else:
                nc.vector.tensor_copy(ot[:, st % GRP, :], pf[:])
            if st % GRP == GRP - 1:
                g = st // GRP
                nc.sync.dma_start(out_pm[:, g * GRP:(g + 1) * GRP, :], ot[:])

    nc.compile()
    return nc


def _host_prep(inputs):
    f64 = np.float64
    Wq = np.asarray(inputs["Wq"], f64)
    bq = np.asarray(inputs["bq"], f64)
    Wk = np.asarray(inputs["Wk"], f64)
    bk = np.asarray(inputs["bk"], f64)
    Wqa = np.asarray(inputs["Wqa"], f64)
    bqa = np.asarray(inputs["bqa"], f64)
    Wka = np.asarray(inputs["Wka"], f64)
    bka = np.asarray(inputs["bka"], f64)
    Wt = np.asarray(inputs["Wt"], f64)
    bt = np.asarray(inputs["bt"], f64)

    c = np.ascontiguousarray
    common = {
        "a1t": c((_SCALE * (Wqa @ Wq)).T.astype(np.float16)),
        "wkast": c((_SCALE * _K2 * Wka).T.astype(np.float32)),
        "wqT16": c(Wq.T.astype(np.float16)),
        "wqT32": c(Wq.T.astype(np.float32)),
        "wqn16": c(Wq.astype(np.float16)),
        "wkn16": c(Wk.astype(np.float16)),
        "wkT16": c(Wk.T.astype(np.float16)),
        "wtTk16": c((_KAPPA * Wt.T).astype(np.float16)),
        "ident": np.eye(128, dtype=np.float16),
        "c1": (_SCALE * (Wqa @ bq + bqa)).astype(np.float32).reshape(_NH, 1),
        "sbka": (_SCALE * bka).astype(np.float32).reshape(_NH, 1),
        "bqhd": bq.astype(np.float32).reshape(_H, 1),
        "bkhd16": bk.astype(np.float16).reshape(_H, 1),
        "bkhd32": bk.astype(np.float32).reshape(_H, 1),
    }
    with_bias_final = bool(np.any(bq != 0) or np.any(bt != 0))
    if with_bias_final:
        common["bq16"] = bq.astype(np.float16).reshape(_H, 1)
        common["bt"] = bt.astype(np.float32).reshape(1, _H)
    return common, with_bias_final


def kernel(**inputs):
    from concourse import bass_utils

    hs = np.asarray(inputs["hidden_states"], np.float32)
    assert hs.shape == (_B, _S, _H), hs.shape

    common, with_bias_final = _host_prep(inputs)
    if with_bias_final not in _BUILT:
        _BUILT[with_bias_final] = _build(with_bias_final)
    nc = _BUILT[with_bias_final]

    in_maps = [dict(common, x=np.ascontiguousarray(hs[b])) for b in range(_B)]
    res = bass_utils.run_bass_kernel_spmd(nc, in_maps, core_ids=list(range(_B)))
    global LAST_RESULTS
    LAST_RESULTS = res
    out = np.stack([r["out"] for r in res.results], axis=0)
    return out.astype(np.float32)


if __name__ == "__main__":
    import sys
    if "--tlsim" in sys.argv:
        # Cost-model timeline estimate of one core's execution.
        from concourse.timeline_sim import TimelineSim
        nc = _build(False)
        tl = TimelineSim(nc, trace="--trace" in sys.argv)
        t = tl.simulate()
        print(f"TimelineSim estimated exec: {t:.0f} ns = {t/1000:.1f} us")
        if tl.perfetto is not None:
            print("perfetto:", tl.perfetto)
    elif "--sim" in sys.argv:
        # CoreSim validation of a single core against the numpy rewrite.
        from concourse.bass_interp import CoreSim
        sys.path.insert(0, "/root/problem")
        from algebra_check import make_inputs, ref_numpy

        inputs = make_inputs()
        common, wbf = _host_prep(inputs)
        nc = _build(wbf)
        sim = CoreSim(nc)
        for k, v in common.items():
            sim.tensor(k)[:] = v
        sim.tensor("x")[:] = inputs["hidden_states"][0]
        sim.simulate(check_with_hw=False)
        got = np.array(sim.tensor("out"))
        ref = ref_numpy(**inputs)[0]
        err = np.abs(got - ref).max()
        print("sim absmax err:", err, "rel-to-scale:", err / np.abs(ref).max())



# revision 4
# speedup vs baseline: 2.4022x; 2.4022x over previous
"""Trainium2 Bass kernel for nn_FastSelfAttention (sparse_attention), v2.

Math (per batch b, x = hidden_states[b]):
    mq = x@Wq.T + bq ; q_w = softmax_S((mq@Wqa.T + bqa)*s)
    pooled_q = einsum(q_w, mq) ; mqk = (x@Wk.T + bk) * pooled_q
    k_w = softmax_S((mqk@Wka.T + bka)*s) ; pooled_k = einsum(k_w, mqk)
    out = (pooled_k * mq)@Wt.T + bt + mq

Algebraic collapse (same as v1):
    q_score = x@A1.T + c1,  A1 = s*Wqa@Wq (host)
    pooled_q[hd] = (softmax-pool of x)[head(hd)] . Wq[hd] + bq[hd]
    A2.T = Wk.T @ ((s*K2*Wka).T * pooled_q)  (device, tiny)
    pooled_k = pooled_q * (xk_pool[h].Wk[hd] + bk[hd])
    W_final = Wq.T @ (pk*Wt.T) + Wq.T ; out = x @ W_final (+ bias row)

v2 data-movement redesign (the v1 bottleneck was serialized DMA):
  - x is loaded fp32 straight into SBUF once; the fp16 transposed copy xT
    is built on-chip (PE fp32 transposes, cast fused into the PSUM->SBUF
    copy). No DRAM round-trips (v1 cast x to fp16 in DRAM, re-read it
    twice, and round-tripped the exp weights).
  - scores are computed in [s-partition, head] orientation (x tiles
    stationary), so exp weights land directly in the layout the pooling
    matmuls consume - no exp transposes - and score/pool matmuls stream
    only 8-wide outputs.
  - pooling contracts x32 (fp32) directly as the stationary operand;
    chunk-local PSUM groups are reduced across chunks on DVE.
  - the front loop is software-pipelined one chunk ahead (transposes of
    chunk c+1 are issued before scores/pools of chunk c) so PE never
    stalls on the cast copies or exp; the k pass runs scores two groups
    ahead of the pools.
  - all big weights load in the DMA-idle window right after x.
Per-core HBM traffic: 8 MB x in + 8 MB out + ~2.6 MB weights (v1: ~32 MB).
"""

import numpy as np

_B, _S, _H, _NH = 8, 4096, 512, 8
_D = _H // _NH
_SCALE = 1.0 / float(np.sqrt(_D))
_K2 = 64.0  # scaling on the A2 path so fp16 entries stay normal

_NT = _S // 128  # 32 sequence tiles
_KT = _H // 128  # 4 feature tiles
# front chunk plan: big chunks first, tapering so the pipeline tail is short
_CHT = [4, 4, 4, 4, 4, 4, 2, 2, 2, 2]
_CH = len(_CHT)
_CT0 = [sum(_CHT[:i]) for i in range(_CH)]
_TPC = 4  # max tiles per chunk (bias tile sizing)

_BUILT = {}
LAST_RESULTS = None
SECTIONS = []


def _build(with_bias):
    import concourse.bacc as bacc
    import concourse.tile as tile
    from concourse import mybir
    from contextlib import ExitStack

    f32 = mybir.dt.float32
    f16 = mybir.dt.float16
    Exp = mybir.ActivationFunctionType.Exp

    nc = bacc.Bacc(
        "TRN2",
        target_bir_lowering=False,
        debug=False,
        enable_asserts=False,
        num_devices=8,
    )

    def din(name, shape, dt=f32):
        return nc.dram_tensor(name, shape, dt, kind="ExternalInput").ap()

    x_d = din("x", [_S, _H])
    # pre32 = [wkast | bqhd | bkhd] packed in partition-tile layout
    pre32_d = din("pre32", [128, 40], f32)
    # pre16 = [a1t | id16]
    pre16_d = din("pre16", [128, 160], f16)
    # packed big weights in partition-tile layout: A = [Wq.T | Wk], B = [Wk.T | Wt.T | Wq]
    wpackA_d = din("wpackA", [128, 2 * _KT, _H], f16)
    wpackB_d = din("wpackB", [128, 3 * _KT, _H], f16)
    if with_bias:
        c1t4_d = din("c1t4", [1, _TPC * _NH], f16)  # tile(s*(Wqa@bq+bqa), 4)
        bkhd16_d = din("bkhd16", [_H, 1], f16)
        sbkat_d = din("sbkat", [1, _NH], f32)       # s*bka
        bq16_d = din("bq16", [_H, 1], f16)
        bt_d = din("bt", [1, _H], f32)
    out_d = nc.dram_tensor("out", [_S, _H], f32, kind="ExternalOutput").ap()

    def mark(name):
        SECTIONS.append((name, sum(1 for _ in nc.all_instructions())))

    with tile.TileContext(nc) as tc, ExitStack() as ctx:
        wpool = ctx.enter_context(tc.tile_pool(name="wpool", bufs=1))
        xpool = ctx.enter_context(tc.tile_pool(name="xpool", bufs=1))
        spool = ctx.enter_context(tc.tile_pool(name="spool", bufs=1))
        opool = ctx.enter_context(tc.tile_pool(name="opool", bufs=4))
        pst = ctx.enter_context(tc.tile_pool(name="pst", bufs=3, space="PSUM"))
        pden = ctx.enter_context(tc.tile_pool(name="pden", bufs=1, space="PSUM"))
        pbig = ctx.enter_context(tc.tile_pool(name="pbig", bufs=4, space="PSUM"))

        def load_w(src, name, eng=None):
            """[H, C] dram -> [128, H//128, C] sbuf (feature tiles on partitions)."""
            t = wpool.tile([128, src.shape[0] // 128, src.shape[1]], src.dtype, name=name)
            (eng or nc.sync).dma_start(t[:], src.rearrange("(t p) c -> p t c", p=128))
            return t

        # --- minimal weights (packed, DMA'd after the first two x chunks)
        pre32 = wpool.tile([128, 40], f32, name="pre32")
        pre16 = wpool.tile([128, 160], f16, name="pre16")
        id16 = pre16[:, 32:160]
        wkast_aps = [pre32[:, h * _NH:(h + 1) * _NH] for h in range(_KT)]
        bqhd = pre32[:, 32:36]
        bkhd = pre32[:, 36:40]
        a1t_aps = [pre16[:, kt * _NH:(kt + 1) * _NH] for kt in range(_KT)]
        ones16 = wpool.tile([128, 1], f16, name="ones16")
        nc.vector.memset(ones16[:], 1.0)
        ones1 = wpool.tile([1, 128], f32, name="ones1")
        nc.vector.memset(ones1[:], 1.0)
        if with_bias:
            onesc = wpool.tile([1, 128], f16, name="onesc")
            nc.vector.memset(onesc[:], 1.0)

        x16 = xpool.tile([128, _NT, _H], f16, name="x16")
        xT = xpool.tile([128, _KT, _S], f16, name="xT")
        expq = spool.tile([128, _NT * _NH], f16, name="expq")
        expk = spool.tile([128, _NT * _NH], f16, name="expk")
        x_pm = x_d.rearrange("(t p) i -> p t i", p=128)

        # copy-engine rotation for PSUM->SBUF traffic (GPSIMD cannot
        # access PSUM, so only Act and DVE qualify)
        def cp(i, out, in_):
            if i % 2 == 0:
                nc.scalar.copy(out, in_)
            else:
                nc.vector.tensor_copy(out, in_)

        accq = spool.tile([128, _KT * _NH], f16, name="accq")
        denq = pden.tile([1, _NH], f32, name="denq", tag="den")
        c1t4 = bkhd16 = sbkat = None

        ncp = 0

        def dma_chunk(c):
            nonlocal c1t4, bkhd16, sbkat
            t0, n = _CT0[c], _CHT[c]
            nc.gpsimd.dma_start(x16[:, t0:t0 + n, :], x_pm[:, t0:t0 + n, :])
            if c == 0 and with_bias:
                c1t4 = wpool.tile([1, _TPC * _NH], f16, name="c1t4")
                nc.sync.dma_start(c1t4[:], c1t4_d[:])
                bkhd16 = load_w(bkhd16_d, "bkhd16")
                sbkat = wpool.tile([1, _NH], f32, name="sbkat")
                nc.sync.dma_start(sbkat[:], sbkat_d[:])

        def xpose_chunk(c):
            nonlocal ncp
            t0, n = _CT0[c], _CHT[c]
            cw = n * 128
            last = (c == _CH - 1)
            for kp in range(2):
                pxp = pbig.tile([128, 2, cw], f16, name=f"xp{c}_{kp}", tag="big")
                for kl in range(2):
                    for tt in range(n):
                        kt = 2 * kp + kl
                        nc.tensor.transpose(
                            pxp[:, kl, tt * 128:(tt + 1) * 128],
                            x16[:, t0 + tt, kt * 128:(kt + 1) * 128],
                            id16,
                        )
                # one full-bank copy per kt-pair: Act and DVE split them;
                # the last chunk uses DVE+Pool so Act can run exp at once
                dst = xT[:, 2 * kp:2 * kp + 2, t0 * 128:t0 * 128 + cw]
                if kp == 0 and not last:
                    nc.scalar.copy(dst, pxp[:])
                else:
                    nc.vector.tensor_copy(dst, pxp[:])

        def score_chunk(c, rhs_aps, bias_rhs, exp_scale, exp_out, tag):
            """scores in [s, head] orientation -> exp."""
            t0, _TPC = _CT0[c], _CHT[c]
            st = pst.tile([128, _TPC * _NH], f32, name=f"st{tag}{c}", tag="st")
            for tt in range(_TPC):
                sl = st[:, tt * _NH:(tt + 1) * _NH]
                for kt in range(_KT):
                    nc.tensor.matmul(
                        sl,
                        xT[:, kt, (t0 + tt) * 128:(t0 + tt + 1) * 128],
                        rhs_aps[kt],
                        start=(kt == 0),
                        stop=(kt == _KT - 1 and bias_rhs is None),
                    )
                if bias_rhs is not None:
                    nc.tensor.matmul(sl, onesc[:], bias_rhs(tt), start=False,
                                     stop=True)
            nc.scalar.activation(
                exp_out[:, t0 * _NH:(t0 + _TPC) * _NH], st[:], Exp,
                scale=exp_scale,
            )

        # --- front: dma / transpose one chunk ahead of scores+pools
        mark("front")
        qbias = (lambda tt: c1t4[:, tt * _NH:(tt + 1) * _NH]) if with_bias else None
        dma_chunk(0)
        dma_chunk(1)
        nc.sync.dma_start(pre32[:], pre32_d[:])
        nc.sync.dma_start(pre16[:], pre16_d[:])
        xpose_chunk(0)
        xpose_chunk(1)
        def qpool_part(acc_ps, trange, cont=False):
            # cont=True: continue accumulating into an already-stopped bank
            # (plain PSUM accumulate; the group checker can't model it)
            for it in range(_KT):
                for i, t in enumerate(trange):
                    nc.tensor.matmul(
                        acc_ps[:, it * _NH:(it + 1) * _NH],
                        x16[:, t, it * 128:(it + 1) * 128],
                        expq[:, t * _NH:(t + 1) * _NH],
                        start=(i == 0 and not cont),
                        stop=(i == len(trange) - 1),
                        skip_group_check=cont,
                    )

        qaccA = qaccB = None
        ntail = _CHT[-1]
        for c in range(_CH):
            if c + 2 < _CH:
                dma_chunk(c + 2)
            score_chunk(c, a1t_aps, qbias, 1.0, expq, "q")
            t0, n = _CT0[c], _CHT[c]
            for tt in range(n):
                nc.tensor.matmul(
                    denq[:], ones16[:],
                    expq[:, (t0 + tt) * _NH:(t0 + tt + 1) * _NH],
                    start=(c == 0 and tt == 0),
                    stop=(c == _CH - 1 and tt == n - 1),
                )
            if c + 2 < _CH:
                xpose_chunk(c + 2)
            if c == _CH - 2:
                # pools for all but the last chunk, while it finishes
                qaccA = pbig.tile([128, _KT * _NH], f32, name="qaccA", tag="big")
                qpool_part(qaccA, range(0, _NT - ntail))

        # --- remaining q pool (last chunk): accumulate into the same bank
        qpool_part(qaccA, range(_NT - ntail, _NT), cont=True)
        nc.vector.tensor_copy(accq[:], qaccA[:])

        # --- big weights in the DMA-idle window right after x
        mark("wload")
        wpackA = wpool.tile([128, 2 * _KT, _H], f16, name="wpackA")
        nc.gpsimd.dma_start(wpackA[:], wpackA_d[:])
        wpackB = wpool.tile([128, 3 * _KT, _H], f16, name="wpackB")
        nc.gpsimd.dma_start(wpackB[:], wpackB_d[:])
        wqT = wpackA[:, 0:_KT]
        wkn = wpackA[:, _KT:2 * _KT]
        wkT = wpackB[:, 0:_KT]
        wtT = wpackB[:, _KT:2 * _KT]
        wqn = wpackB[:, 2 * _KT:3 * _KT]

        def den_rec(dps, pfx):
            """den [1,h8] psum -> precS [128, NH] sbuf (1/den per column)."""
            rec = spool.tile([1, _NH], f32, name=f"{pfx}_rec")
            nc.vector.reciprocal(rec[:], dps[:])
            prec = pst.tile([128, _NH], f32, name=f"{pfx}_prec", tag="st")
            nc.tensor.matmul(prec[:], ones1[:], rec[:], start=True, stop=True)
            precS = spool.tile([128, _NH], f32, name=f"{pfx}_precS")
            nc.vector.tensor_copy(precS[:], prec[:])
            return precS

        def pooled_vec(wT, acc, prec, badd, pfx):
            """pv[hd] = rec[head(hd)] * (unnorm pool[head(hd)] . W[hd,:])
            (+ b[hd]) -> [128, KT] f32.  Normalization fused into the
            diagonal extraction."""
            pm = pst.tile([128, _KT * _NH], f32, name=f"{pfx}_pm", tag="st")
            for it in range(_KT):
                for kt in range(_KT):
                    nc.tensor.matmul(
                        pm[:, it * _NH:(it + 1) * _NH],
                        wT[:, kt, it * 128:(it + 1) * 128],
                        acc[:, kt * _NH:(kt + 1) * _NH],
                        start=(kt == 0),
                        stop=(kt == _KT - 1),
                    )
            pv = spool.tile([128, _KT], f32, name=f"{pfx}_pv")
            for half in range(2):
                sl = slice(64 * half, 64 * (half + 1))
                src_ap = pm[sl, half:half + 31:10]
                rec_ap = prec[sl, half:half + 7:2]
                nc.vector.tensor_mul(pv[sl, :], src_ap, rec_ap)
                if with_bias:
                    nc.vector.tensor_add(pv[sl, :], pv[sl, :], badd[sl, :])
            return pv

        # --- q path tail
        mark("qtail")
        precq = den_rec(denq, "q")
        pq = pooled_vec(wqT, accq, precq, bqhd, "pq")

        # --- A2.T = Wk.T @ (wkast * pq)
        wkapq = spool.tile([128, _KT, _NH], f16, name="wkapq")
        for ht in range(_KT):
            nc.vector.tensor_scalar_mul(
                wkapq[:, ht, :], wkast_aps[ht], pq[:, ht:ht + 1]
            )
        pa2 = pst.tile([128, _KT * _NH], f32, name="pa2", tag="st")
        for it in range(_KT):
            for ht in range(_KT):
                nc.tensor.matmul(
                    pa2[:, it * _NH:(it + 1) * _NH],
                    wkn[:, ht, it * 128:(it + 1) * 128],
                    wkapq[:, ht, :],
                    start=(ht == 0),
                    stop=(ht == _KT - 1),
                )
        a2T = spool.tile([128, _KT, _NH], f16, name="a2T")
        nc.vector.tensor_copy(a2T[:, :, :], pa2[:])
        if with_bias:
            pc2 = pst.tile([1, _NH], f32, name="pc2", tag="st")
            for ht in range(_KT):
                nc.tensor.matmul(pc2[:], bkhd16[:, ht, :], wkapq[:, ht, :],
                                 start=(ht == 0), stop=(ht == _KT - 1))
            c2t = spool.tile([1, _NH], f32, name="c2t")
            nc.scalar.mul(c2t[:], pc2[:], 1.0 / _K2)
            c2bT = spool.tile([1, _NH], f16, name="c2bT")
            nc.vector.tensor_add(c2bT[:], c2t[:], sbkat[:])

        # --- k pass (xT resident; no DMA): all 32 score tiles fit one
        # psum bank ([128, 256] f32, sequential accumulation groups), so the
        # whole pass is scores -> ONE exp -> dens -> pools with no chaining.
        mark("kpass")
        kscore = pbig.tile([128, _NT * _NH], f32, name="kscore", tag="big")
        for t in range(_NT):
            sl = kscore[:, t * _NH:(t + 1) * _NH]
            for kt in range(_KT):
                nc.tensor.matmul(
                    sl,
                    xT[:, kt, t * 128:(t + 1) * 128],
                    a2T[:, kt, :],
                    start=(kt == 0),
                    stop=(kt == _KT - 1 and not with_bias),
                )
            if with_bias:
                nc.tensor.matmul(sl, onesc[:], c2bT[:], start=False, stop=True)
        nc.scalar.activation(expk[:], kscore[:], Exp, scale=1.0 / _K2)
        denk = pden.tile([1, _NH], f32, name="denk", tag="den")
        for t in range(_NT):
            nc.tensor.matmul(
                denk[:], ones16[:], expk[:, t * _NH:(t + 1) * _NH],
                start=(t == 0), stop=(t == _NT - 1),
            )
        kacc = pbig.tile([128, _KT * _NH], f32, name="kacc", tag="big")
        for it in range(_KT):
            for t in range(_NT):
                nc.tensor.matmul(
                    kacc[:, it * _NH:(it + 1) * _NH],
                    x16[:, t, it * 128:(it + 1) * 128],
                    expk[:, t * _NH:(t + 1) * _NH],
                    start=(t == 0),
                    stop=(t == _NT - 1),
                )
        acck = spool.tile([128, _KT * _NH], f16, name="acck")
        nc.vector.tensor_copy(acck[:], kacc[:])

        mark("ktail")
        preck = den_rec(denk, "k")
        prek = pooled_vec(wkT, acck, preck, bkhd, "pk")
        pk = spool.tile([128, _KT], f32, name="pk")
        nc.vector.tensor_mul(pk[:], prek[:], pq[:])

        # --- W_final = Wq.T @ (pk * Wt.T) + Wq.T
        m1 = spool.tile([128, _KT, _H], f16, name="m1")
        for jt in range(_KT):
            nc.vector.tensor_scalar_mul(m1[:, jt, :], wtT[:, jt, :],
                                        pk[:, jt:jt + 1])
        wf = spool.tile([128, _KT, _H], f16, name="wf")
        for it in range(_KT):
            pw = pbig.tile([128, _H], f32, name=f"pw{it}", tag="big")
            for jt in range(_KT):
                nc.tensor.matmul(
                    pw[:],
                    wqn[:, jt, it * 128:(it + 1) * 128],
                    m1[:, jt, :],
                    start=(jt == 0),
                    stop=(jt == _KT - 1),
                )
            nc.vector.tensor_add(wf[:, it, :], pw[:], wqT[:, it, :])

        if with_bias:
            bq16 = load_w(bq16_d, "bq16")
            bt_sb = wpool.tile([1, _H], f32, name="bt_sb")
            nc.sync.dma_start(bt_sb[:], bt_d[:])
            pbf = pst.tile([1, _H], f32, name="pbf", tag="st")
            for jt in range(_KT):
                nc.tensor.matmul(pbf[:], bq16[:, jt, :], m1[:, jt, :],
                                 start=(jt == 0), stop=(jt == _KT - 1))
            bf16 = spool.tile([1, _H], f16, name="bf16")
            nc.vector.tensor_add(bf16[:], pbf[:], bt_sb[:])
            one_row = spool.tile([1, 128], f16, name="one_row")
            nc.vector.memset(one_row[:], 1.0)

        # --- final: out = x @ W_final (+ bias row)
        mark("final")
        out_pm = out_d.rearrange("(t p) m -> p t m", p=128)
        GRP = 2
        for st in range(_NT):
            if st % GRP == 0:
                ot = opool.tile([128, GRP, _H], f32, name=f"ot{st}", tag="ot",
                                bufs=4)
            pf = pbig.tile([128, _H], f32, name=f"pf{st}", tag="big")
            for it in range(_KT):
                nc.tensor.matmul(
                    pf[:],
                    xT[:, it, st * 128:(st + 1) * 128],
                    wf[:, it, :],
                    start=(it == 0),
                    stop=(it == _KT - 1 and not with_bias),
                )
            if with_bias:
                nc.tensor.matmul(pf[:], one_row[:], bf16[:], start=False, stop=True)
            cp(ncp, ot[:, st % GRP, :], pf[:])
            ncp += 1
            if st % GRP == GRP - 1:
                g = st // GRP
                nc.sync.dma_start(out_pm[:, g * GRP:(g + 1) * GRP, :], ot[:])

    mark("end")
    nc.compile()
    return nc


def _host_prep(inputs):
    f64 = np.float64
    Wq = np.asarray(inputs["Wq"], f64)
    bq = np.asarray(inputs["bq"], f64)
    Wk = np.asarray(inputs["Wk"], f64)
    bk = np.asarray(inputs["bk"], f64)
    Wqa = np.asarray(inputs["Wqa"], f64)
    bqa = np.asarray(inputs["bqa"], f64)
    Wka = np.asarray(inputs["Wka"], f64)
    bka = np.asarray(inputs["bka"], f64)
    Wt = np.asarray(inputs["Wt"], f64)
    bt = np.asarray(inputs["bt"], f64)

    c = np.ascontiguousarray

    def ptile(a):
        # [H, C] -> [128, (H//128)*C] partition-tile packing (load_w layout)
        return a.reshape(_KT, 128, -1).transpose(1, 0, 2).reshape(128, -1)

    a1tP = ptile((_SCALE * (Wqa @ Wq)).T.astype(np.float16))
    wkastP = ptile((_SCALE * _K2 * Wka).T.astype(np.float32))
    pre32 = np.concatenate(
        [wkastP,
         ptile(bq.astype(np.float32).reshape(_H, 1)),
         ptile(bk.astype(np.float32).reshape(_H, 1))], axis=1)
    pre16 = np.concatenate([a1tP, np.eye(128, dtype=np.float16)], axis=1)
    def ptile3(a):
        return a.reshape(_KT, 128, _H).transpose(1, 0, 2)

    wpackA = np.concatenate(
        [ptile3(Wq.T.astype(np.float16)), ptile3(Wk.astype(np.float16))], axis=1)
    wpackB = np.concatenate(
        [ptile3(Wk.T.astype(np.float16)), ptile3(Wt.T.astype(np.float16)),
         ptile3(Wq.astype(np.float16))], axis=1)
    common = {
        "pre32": c(pre32),
        "pre16": c(pre16),
        "wpackA": c(wpackA),
        "wpackB": c(wpackB),
    }
    with_bias = bool(
        np.any(bq != 0) or np.any(bk != 0) or np.any(bqa != 0)
        or np.any(bka != 0) or np.any(bt != 0)
    )
    if with_bias:
        c1 = (_SCALE * (Wqa @ bq + bqa)).astype(np.float16)
        common["c1t4"] = np.tile(c1, _TPC).reshape(1, _TPC * _NH)
        common["bkhd16"] = bk.astype(np.float16).reshape(_H, 1)
        common["sbkat"] = (_SCALE * bka).astype(np.float32).reshape(1, _NH)
        common["bq16"] = bq.astype(np.float16).reshape(_H, 1)
        common["bt"] = bt.astype(np.float32).reshape(1, _H)
    return common, with_bias


def kernel(**inputs):
    from concourse import bass_utils

    hs = np.asarray(inputs["hidden_states"], np.float32)
    assert hs.shape == (_B, _S, _H), hs.shape

    common, with_bias = _host_prep(inputs)
    if with_bias not in _BUILT:
        _BUILT[with_bias] = _build(with_bias)
    nc = _BUILT[with_bias]

    in_maps = [dict(common, x=np.ascontiguousarray(hs[b])) for b in range(_B)]
    res = bass_utils.run_bass_kernel_spmd(nc, in_maps, core_ids=list(range(_B)))
    global LAST_RESULTS
    LAST_RESULTS = res
    out = np.stack([r["out"] for r in res.results], axis=0)
    return out.astype(np.float32)


if __name__ == "__main__":
    import sys
    if "--tlsim" in sys.argv:
        from concourse.timeline_sim import TimelineSim
        nc = _build("--bias" in sys.argv)
        tl = TimelineSim(nc, trace=False)
        t = tl.simulate()
        print(f"TimelineSim estimated exec: {t:.0f} ns = {t/1000:.1f} us")
    elif "--sim" in sys.argv:
        from concourse.bass_interp import CoreSim
        sys.path.insert(0, "/root/problem")
        from algebra_check import make_inputs, ref_numpy

        inputs = make_inputs()
        if "--bias" in sys.argv:
            rng = np.random.default_rng(7)
            for k in ("bq", "bk", "bt"):
                inputs[k] = (rng.standard_normal(_H) * 0.02).astype(np.float32)
            for k in ("bqa", "bka"):
                inputs[k] = (rng.standard_normal(_NH) * 0.02).astype(np.float32)
        common, wb = _host_prep(inputs)
        print("with_bias =", wb)
        nc = _build(wb)
        sim = CoreSim(nc)
        for k, v in common.items():
            sim.tensor(k)[:] = v
        sim.tensor("x")[:] = inputs["hidden_states"][0]
        sim.simulate(check_with_hw=False)
        got = np.array(sim.tensor("out"))
        ref = ref_numpy(**inputs)[0]
        err = np.abs(got - ref).max()
        print("sim absmax err:", err, "rel-to-scale:", err / np.abs(ref).max())


# revision 7
# speedup vs baseline: 2.4188x; 1.0069x over previous
"""Trainium2 Bass kernel for nn_FastSelfAttention (sparse_attention), v2.

Math (per batch b, x = hidden_states[b]):
    mq = x@Wq.T + bq ; q_w = softmax_S((mq@Wqa.T + bqa)*s)
    pooled_q = einsum(q_w, mq) ; mqk = (x@Wk.T + bk) * pooled_q
    k_w = softmax_S((mqk@Wka.T + bka)*s) ; pooled_k = einsum(k_w, mqk)
    out = (pooled_k * mq)@Wt.T + bt + mq

Algebraic collapse (same as v1):
    q_score = x@A1.T + c1,  A1 = s*Wqa@Wq (host)
    pooled_q[hd] = (softmax-pool of x)[head(hd)] . Wq[hd] + bq[hd]
    A2.T = Wk.T @ ((s*K2*Wka).T * pooled_q)  (device, tiny)
    pooled_k = pooled_q * (xk_pool[h].Wk[hd] + bk[hd])
    W_final = Wq.T @ (pk*Wt.T) + Wq.T ; out = x @ W_final (+ bias row)

v2 data-movement redesign (the v1 bottleneck was serialized DMA):
  - x is loaded fp32 straight into SBUF once; the fp16 transposed copy xT
    is built on-chip (PE fp32 transposes, cast fused into the PSUM->SBUF
    copy). No DRAM round-trips (v1 cast x to fp16 in DRAM, re-read it
    twice, and round-tripped the exp weights).
  - scores are computed in [s-partition, head] orientation (x tiles
    stationary), so exp weights land directly in the layout the pooling
    matmuls consume - no exp transposes - and score/pool matmuls stream
    only 8-wide outputs.
  - pooling contracts x32 (fp32) directly as the stationary operand;
    chunk-local PSUM groups are reduced across chunks on DVE.
  - the front loop is software-pipelined one chunk ahead (transposes of
    chunk c+1 are issued before scores/pools of chunk c) so PE never
    stalls on the cast copies or exp; the k pass runs scores two groups
    ahead of the pools.
  - all big weights load in the DMA-idle window right after x.
Per-core HBM traffic: 8 MB x in + 8 MB out + ~2.6 MB weights (v1: ~32 MB).
"""

import numpy as np

_B, _S, _H, _NH = 8, 4096, 512, 8
_D = _H // _NH
_SCALE = 1.0 / float(np.sqrt(_D))
_K2 = 64.0  # scaling on the A2 path so fp16 entries stay normal

_NT = _S // 128  # 32 sequence tiles
_KT = _H // 128  # 4 feature tiles
# front chunk plan: big chunks first, tapering so the pipeline tail is short
_CHT = [4, 4, 4, 4, 4, 4, 2, 2, 2, 2]
_CH = len(_CHT)
_CT0 = [sum(_CHT[:i]) for i in range(_CH)]
_TPC = 4  # max tiles per chunk (bias tile sizing)

_BUILT = {}
LAST_RESULTS = None
SECTIONS = []


def _build(with_bias):
    import concourse.bacc as bacc
    import concourse.tile as tile
    from concourse import mybir
    from contextlib import ExitStack

    f32 = mybir.dt.float32
    f16 = mybir.dt.float16
    Exp = mybir.ActivationFunctionType.Exp

    nc = bacc.Bacc(
        "TRN2",
        target_bir_lowering=False,
        debug=False,
        enable_asserts=False,
        num_devices=8,
    )

    def din(name, shape, dt=f32):
        return nc.dram_tensor(name, shape, dt, kind="ExternalInput").ap()

    x_d = din("x", [_S, _H])
    # pre32 = [wkast | bqhd | bkhd] packed in partition-tile layout
    pre32_d = din("pre32", [128, 40], f32)
    # pre16 = [a1t | id16]
    pre16_d = din("pre16", [128, 160], f16)
    # packed big weights in partition-tile layout: A = [Wq.T | Wk], B = [Wk.T | Wt.T | Wq]
    wpackA_d = din("wpackA", [128, 2 * _KT, _H], f16)
    wpackB_d = din("wpackB", [128, 3 * _KT, _H], f16)
    if with_bias:
        c1t4_d = din("c1t4", [1, _TPC * _NH], f16)  # tile(s*(Wqa@bq+bqa), 4)
        bkhd16_d = din("bkhd16", [_H, 1], f16)
        sbkat_d = din("sbkat", [1, _NH], f32)       # s*bka
        bq16_d = din("bq16", [_H, 1], f16)
        bt_d = din("bt", [1, _H], f32)
    out_d = nc.dram_tensor("out", [_S, _H], f32, kind="ExternalOutput").ap()

    def mark(name):
        SECTIONS.append((name, sum(1 for _ in nc.all_instructions())))

    with tile.TileContext(nc) as tc, ExitStack() as ctx:
        wpool = ctx.enter_context(tc.tile_pool(name="wpool", bufs=1))
        xpool = ctx.enter_context(tc.tile_pool(name="xpool", bufs=1))
        spool = ctx.enter_context(tc.tile_pool(name="spool", bufs=1))
        opool = ctx.enter_context(tc.tile_pool(name="opool", bufs=4))
        pst = ctx.enter_context(tc.tile_pool(name="pst", bufs=2, space="PSUM"))
        pden = ctx.enter_context(tc.tile_pool(name="pden", bufs=1, space="PSUM"))
        pbig = ctx.enter_context(tc.tile_pool(name="pbig", bufs=4, space="PSUM"))
        pacc = ctx.enter_context(tc.tile_pool(name="pacc", bufs=1, space="PSUM"))

        def load_w(src, name, eng=None):
            """[H, C] dram -> [128, H//128, C] sbuf (feature tiles on partitions)."""
            t = wpool.tile([128, src.shape[0] // 128, src.shape[1]], src.dtype, name=name)
            (eng or nc.sync).dma_start(t[:], src.rearrange("(t p) c -> p t c", p=128))
            return t

        # --- minimal weights (packed, DMA'd after the first two x chunks)
        pre32 = wpool.tile([128, 40], f32, name="pre32")
        pre16 = wpool.tile([128, 160], f16, name="pre16")
        id16 = pre16[:, 32:160]
        wkast_aps = [pre32[:, h * _NH:(h + 1) * _NH] for h in range(_KT)]
        bqhd = pre32[:, 32:36]
        bkhd = pre32[:, 36:40]
        a1t_aps = [pre16[:, kt * _NH:(kt + 1) * _NH] for kt in range(_KT)]
        ones16 = wpool.tile([128, 1], f16, name="ones16")
        nc.vector.memset(ones16[:], 1.0)
        ones1 = wpool.tile([1, 128], f32, name="ones1")
        nc.vector.memset(ones1[:], 1.0)
        if with_bias:
            onesc = wpool.tile([1, 128], f16, name="onesc")
            nc.vector.memset(onesc[:], 1.0)

        x16 = xpool.tile([128, _NT, _H], f16, name="x16")
        xT = xpool.tile([128, _KT, _S], f16, name="xT")
        expq = spool.tile([128, _NT * _NH], f16, name="expq")
        expk = spool.tile([128, _NT * _NH], f16, name="expk")
        x_pm = x_d.rearrange("(t p) i -> p t i", p=128)

        # copy-engine rotation for PSUM->SBUF traffic (GPSIMD cannot
        # access PSUM, so only Act and DVE qualify)
        def cp(i, out, in_):
            if i % 2 == 0:
                nc.scalar.copy(out, in_)
            else:
                nc.vector.tensor_copy(out, in_)

        accq = spool.tile([128, _KT * _NH], f16, name="accq")
        denq = pden.tile([1, _NH], f32, name="denq", tag="den")
        c1t4 = bkhd16 = sbkat = None

        ncp = 0

        def dma_chunk(c):
            nonlocal c1t4, bkhd16, sbkat
            t0, n = _CT0[c], _CHT[c]
            nc.gpsimd.dma_start(x16[:, t0:t0 + n, :], x_pm[:, t0:t0 + n, :])
            if c == 0 and with_bias:
                c1t4 = wpool.tile([1, _TPC * _NH], f16, name="c1t4")
                nc.sync.dma_start(c1t4[:], c1t4_d[:])
                bkhd16 = load_w(bkhd16_d, "bkhd16")
                sbkat = wpool.tile([1, _NH], f32, name="sbkat")
                nc.sync.dma_start(sbkat[:], sbkat_d[:])

        def xpose_chunk(c):
            nonlocal ncp
            t0, n = _CT0[c], _CHT[c]
            cw = n * 128
            last = (c == _CH - 1)
            for kp in range(2):
                pxp = pbig.tile([128, 2, cw], f16, name=f"xp{c}_{kp}", tag="big")
                for kl in range(2):
                    for tt in range(n):
                        kt = 2 * kp + kl
                        nc.tensor.transpose(
                            pxp[:, kl, tt * 128:(tt + 1) * 128],
                            x16[:, t0 + tt, kt * 128:(kt + 1) * 128],
                            id16,
                        )
                # one full-bank copy per kt-pair: Act and DVE split them;
                # the last chunk uses DVE+Pool so Act can run exp at once
                dst = xT[:, 2 * kp:2 * kp + 2, t0 * 128:t0 * 128 + cw]
                if kp == 0 and not last:
                    nc.scalar.copy(dst, pxp[:])
                else:
                    nc.vector.tensor_copy(dst, pxp[:])

        def score_chunk(c, rhs_aps, bias_rhs, exp_scale, exp_out, tag):
            """scores in [s, head] orientation -> exp."""
            t0, _TPC = _CT0[c], _CHT[c]
            st = pst.tile([128, _TPC * _NH], f32, name=f"st{tag}{c}", tag="st")
            for tt in range(_TPC):
                sl = st[:, tt * _NH:(tt + 1) * _NH]
                for kt in range(_KT):
                    nc.tensor.matmul(
                        sl,
                        xT[:, kt, (t0 + tt) * 128:(t0 + tt + 1) * 128],
                        rhs_aps[kt],
                        start=(kt == 0),
                        stop=(kt == _KT - 1 and bias_rhs is None),
                    )
                if bias_rhs is not None:
                    nc.tensor.matmul(sl, onesc[:], bias_rhs(tt), start=False,
                                     stop=True)
            nc.scalar.activation(
                exp_out[:, t0 * _NH:(t0 + _TPC) * _NH], st[:], Exp,
                scale=exp_scale,
            )

        # --- front: dma / transpose one chunk ahead of scores+pools
        mark("front")
        qbias = (lambda tt: c1t4[:, tt * _NH:(tt + 1) * _NH]) if with_bias else None
        dma_chunk(0)
        dma_chunk(1)
        nc.sync.dma_start(pre32[:], pre32_d[:])
        nc.sync.dma_start(pre16[:], pre16_d[:])
        xpose_chunk(0)
        xpose_chunk(1)
        def den_rec(dps, pfx):
            """den [1,h8] psum -> precS [128, NH] sbuf (1/den per column)."""
            rec = spool.tile([1, _NH], f32, name=f"{pfx}_rec")
            nc.vector.reciprocal(rec[:], dps[:])
            prec = pst.tile([128, _NH], f32, name=f"{pfx}_prec", tag="st")
            nc.tensor.matmul(prec[:], ones1[:], rec[:], start=True, stop=True)
            precS = spool.tile([128, _NH], f32, name=f"{pfx}_precS")
            nc.vector.tensor_copy(precS[:], prec[:])
            return precS

        def qpool_part(acc_ps, trange, cont=False):
            # cont=True: continue accumulating into an already-stopped bank
            # (plain PSUM accumulate; the group checker can't model it)
            for it in range(_KT):
                for i, t in enumerate(trange):
                    nc.tensor.matmul(
                        acc_ps[:, it * _NH:(it + 1) * _NH],
                        x16[:, t, it * 128:(it + 1) * 128],
                        expq[:, t * _NH:(t + 1) * _NH],
                        start=(i == 0 and not cont),
                        stop=(i == len(trange) - 1),
                        skip_group_check=cont,
                    )

        qacc = pacc.tile([128, _KT * _NH], f32, name="qacc", tag="acc")
        for c in range(_CH):
            if c + 2 < _CH:
                dma_chunk(c + 2)
            if c > 0:
                # pools for the PREVIOUS chunk (its exp is long done, so these
                # never stall the loop); later chunks continue the psum adds
                t0p, np_ = _CT0[c - 1], _CHT[c - 1]
                qpool_part(qacc, range(t0p, t0p + np_), cont=(c > 1))
            score_chunk(c, a1t_aps, qbias, 1.0, expq, "q")
            t0, n = _CT0[c], _CHT[c]
            for tt in range(n):
                nc.tensor.matmul(
                    denq[:], ones16[:],
                    expq[:, (t0 + tt) * _NH:(t0 + tt + 1) * _NH],
                    start=(c == 0 and tt == 0),
                    stop=(c == _CH - 1 and tt == n - 1),
                )
            if c + 2 < _CH:
                xpose_chunk(c + 2)

        t0, n = _CT0[-1], _CHT[-1]
        precq = den_rec(denq, "q")
        qpool_part(qacc, range(t0, t0 + n), cont=True)
        nc.vector.tensor_copy(accq[:], qacc[:])

        # --- big weights in the DMA-idle window right after x
        mark("wload")
        wpackA = wpool.tile([128, 2 * _KT, _H], f16, name="wpackA")
        nc.gpsimd.dma_start(wpackA[:], wpackA_d[:])
        wpackB = wpool.tile([128, 3 * _KT, _H], f16, name="wpackB")
        nc.gpsimd.dma_start(wpackB[:], wpackB_d[:])
        wqT = wpackA[:, 0:_KT]
        wkn = wpackA[:, _KT:2 * _KT]
        wkT = wpackB[:, 0:_KT]
        wtT = wpackB[:, _KT:2 * _KT]
        wqn = wpackB[:, 2 * _KT:3 * _KT]

        def pooled_vec(wT, acc, prec, badd, pfx):
            """pv[hd] = rec[head(hd)] * (unnorm pool[head(hd)] . W[hd,:])
            (+ b[hd]) -> [128, KT] f32.  Normalization fused into the
            diagonal extraction."""
            pm = pst.tile([128, _KT * _NH], f32, name=f"{pfx}_pm", tag="st")
            for it in range(_KT):
                for kt in range(_KT):
                    nc.tensor.matmul(
                        pm[:, it * _NH:(it + 1) * _NH],
                        wT[:, kt, it * 128:(it + 1) * 128],
                        acc[:, kt * _NH:(kt + 1) * _NH],
                        start=(kt == 0),
                        stop=(kt == _KT - 1),
                    )
            pv = spool.tile([128, _KT], f32, name=f"{pfx}_pv")
            for half in range(2):
                sl = slice(64 * half, 64 * (half + 1))
                src_ap = pm[sl, half:half + 31:10]
                rec_ap = prec[sl, half:half + 7:2]
                nc.vector.tensor_mul(pv[sl, :], src_ap, rec_ap)
                if with_bias:
                    nc.vector.tensor_add(pv[sl, :], pv[sl, :], badd[sl, :])
            return pv

        # --- q path tail
        mark("qtail")
        pq = pooled_vec(wqT, accq, precq, bqhd, "pq")

        # --- A2.T = Wk.T @ (wkast * pq)
        wkapq = spool.tile([128, _KT, _NH], f16, name="wkapq")
        for ht in range(_KT):
            nc.vector.tensor_scalar_mul(
                wkapq[:, ht, :], wkast_aps[ht], pq[:, ht:ht + 1]
            )
        pa2 = pst.tile([128, _KT * _NH], f32, name="pa2", tag="st")
        for it in range(_KT):
            for ht in range(_KT):
                nc.tensor.matmul(
                    pa2[:, it * _NH:(it + 1) * _NH],
                    wkn[:, ht, it * 128:(it + 1) * 128],
                    wkapq[:, ht, :],
                    start=(ht == 0),
                    stop=(ht == _KT - 1),
                )
        a2T = spool.tile([128, _KT, _NH], f16, name="a2T")
        nc.vector.tensor_copy(a2T[:, :, :], pa2[:])
        if with_bias:
            # bias lands in the psum BEFORE the 1/K2 exp scale, so it must
            # carry the K2 factor: c2bT*1/K2 = pc2/K2 + s*bka  (sbkat is
            # host-scaled by K2)
            pc2 = pst.tile([1, _NH], f32, name="pc2", tag="st")
            for ht in range(_KT):
                nc.tensor.matmul(pc2[:], bkhd16[:, ht, :], wkapq[:, ht, :],
                                 start=(ht == 0), stop=(ht == _KT - 1))
            c2bT = spool.tile([1, _NH], f16, name="c2bT")
            nc.vector.tensor_add(c2bT[:], pc2[:], sbkat[:])

        # --- k pass (xT resident; no DMA): all 32 score tiles fit one
        # psum bank ([128, 256] f32, sequential accumulation groups), so the
        # whole pass is scores -> ONE exp -> dens -> pools with no chaining.
        mark("kpass")
        kscore = pbig.tile([128, _NT * _NH], f32, name="kscore", tag="big")
        for t in range(_NT):
            sl = kscore[:, t * _NH:(t + 1) * _NH]
            for kt in range(_KT):
                nc.tensor.matmul(
                    sl,
                    xT[:, kt, t * 128:(t + 1) * 128],
                    a2T[:, kt, :],
                    start=(kt == 0),
                    stop=(kt == _KT - 1 and not with_bias),
                )
            if with_bias:
                nc.tensor.matmul(sl, onesc[:], c2bT[:], start=False, stop=True)
        nc.scalar.activation(expk[:], kscore[:], Exp, scale=1.0 / _K2)
        denk = pden.tile([1, _NH], f32, name="denk", tag="den")
        for t in range(_NT):
            nc.tensor.matmul(
                denk[:], ones16[:], expk[:, t * _NH:(t + 1) * _NH],
                start=(t == 0), stop=(t == _NT - 1),
            )
        kacc = pacc.tile([128, _KT * _NH], f32, name="kacc", tag="acc")
        for it in range(_KT):
            for t in range(_NT):
                nc.tensor.matmul(
                    kacc[:, it * _NH:(it + 1) * _NH],
                    x16[:, t, it * 128:(it + 1) * 128],
                    expk[:, t * _NH:(t + 1) * _NH],
                    start=(t == 0),
                    stop=(t == _NT - 1),
                )
        acck = spool.tile([128, _KT * _NH], f16, name="acck")
        nc.vector.tensor_copy(acck[:], kacc[:])

        mark("ktail")
        preck = den_rec(denk, "k")
        prek = pooled_vec(wkT, acck, preck, bkhd, "pk")
        pk = spool.tile([128, _KT], f32, name="pk")
        nc.vector.tensor_mul(pk[:], prek[:], pq[:])

        # --- W_final = Wq.T @ (pk * Wt.T) + Wq.T
        m1 = spool.tile([128, _KT, _H], f16, name="m1")
        for jt in range(_KT):
            nc.vector.tensor_scalar_mul(m1[:, jt, :], wtT[:, jt, :],
                                        pk[:, jt:jt + 1])
        wf = spool.tile([128, _KT, _H], f16, name="wf")
        for it in range(_KT):
            pw = pbig.tile([128, _H], f32, name=f"pw{it}", tag="big")
            for jt in range(_KT):
                nc.tensor.matmul(
                    pw[:],
                    wqn[:, jt, it * 128:(it + 1) * 128],
                    m1[:, jt, :],
                    start=(jt == 0),
                    stop=(jt == _KT - 1),
                )
            nc.vector.tensor_add(wf[:, it, :], pw[:], wqT[:, it, :])

        if with_bias:
            bq16 = load_w(bq16_d, "bq16")
            bt_sb = wpool.tile([1, _H], f32, name="bt_sb")
            nc.sync.dma_start(bt_sb[:], bt_d[:])
            pbf = pst.tile([1, _H], f32, name="pbf", tag="st")
            for jt in range(_KT):
                nc.tensor.matmul(pbf[:], bq16[:, jt, :], m1[:, jt, :],
                                 start=(jt == 0), stop=(jt == _KT - 1))
            bf16 = spool.tile([1, _H], f16, name="bf16")
            nc.vector.tensor_add(bf16[:], pbf[:], bt_sb[:])
            one_row = spool.tile([1, 128], f16, name="one_row")
            nc.vector.memset(one_row[:], 1.0)

        # --- final: out = x @ W_final (+ bias row)
        mark("final")
        out_pm = out_d.rearrange("(t p) m -> p t m", p=128)
        # pairs of tiles per out DMA, singles for the last two (shorter tail)
        groups = [(s, 2) for s in range(0, _NT - 2, 2)] + [(_NT - 2, 1), (_NT - 1, 1)]
        for g0, gn in groups:
            ot = opool.tile([128, gn, _H], f32, name=f"ot{g0}", tag="ot", bufs=4)
            for st in range(g0, g0 + gn):
                pf = pbig.tile([128, _H], f32, name=f"pf{st}", tag="big")
                for it in range(_KT):
                    nc.tensor.matmul(
                        pf[:],
                        xT[:, it, st * 128:(st + 1) * 128],
                        wf[:, it, :],
                        start=(it == 0),
                        stop=(it == _KT - 1 and not with_bias),
                    )
                if with_bias:
                    nc.tensor.matmul(pf[:], one_row[:], bf16[:], start=False,
                                     stop=True)
                cp(ncp, ot[:, st - g0, :], pf[:])
                ncp += 1
            nc.sync.dma_start(out_pm[:, g0:g0 + gn, :], ot[:])

    mark("end")
    nc.compile()
    return nc


def _host_prep(inputs):
    f64 = np.float64
    Wq = np.asarray(inputs["Wq"], f64)
    bq = np.asarray(inputs["bq"], f64)
    Wk = np.asarray(inputs["Wk"], f64)
    bk = np.asarray(inputs["bk"], f64)
    Wqa = np.asarray(inputs["Wqa"], f64)
    bqa = np.asarray(inputs["bqa"], f64)
    Wka = np.asarray(inputs["Wka"], f64)
    bka = np.asarray(inputs["bka"], f64)
    Wt = np.asarray(inputs["Wt"], f64)
    bt = np.asarray(inputs["bt"], f64)

    c = np.ascontiguousarray

    def ptile(a):
        # [H, C] -> [128, (H//128)*C] partition-tile packing (load_w layout)
        return a.reshape(_KT, 128, -1).transpose(1, 0, 2).reshape(128, -1)

    a1tP = ptile((_SCALE * (Wqa @ Wq)).T.astype(np.float16))
    wkastP = ptile((_SCALE * _K2 * Wka).T.astype(np.float32))
    pre32 = np.concatenate(
        [wkastP,
         ptile(bq.astype(np.float32).reshape(_H, 1)),
         ptile(bk.astype(np.float32).reshape(_H, 1))], axis=1)
    pre16 = np.concatenate([a1tP, np.eye(128, dtype=np.float16)], axis=1)
    def ptile3(a):
        return a.reshape(_KT, 128, _H).transpose(1, 0, 2)

    wpackA = np.concatenate(
        [ptile3(Wq.T.astype(np.float16)), ptile3(Wk.astype(np.float16))], axis=1)
    wpackB = np.concatenate(
        [ptile3(Wk.T.astype(np.float16)), ptile3(Wt.T.astype(np.float16)),
         ptile3(Wq.astype(np.float16))], axis=1)
    common = {
        "pre32": c(pre32),
        "pre16": c(pre16),
        "wpackA": c(wpackA),
        "wpackB": c(wpackB),
    }
    with_bias = bool(
        np.any(bq != 0) or np.any(bk != 0) or np.any(bqa != 0)
        or np.any(bka != 0) or np.any(bt != 0)
    )
    if with_bias:
        c1 = (_SCALE * (Wqa @ bq + bqa)).astype(np.float16)
        common["c1t4"] = np.tile(c1, _TPC).reshape(1, _TPC * _NH)
        common["bkhd16"] = bk.astype(np.float16).reshape(_H, 1)
        common["sbkat"] = (_K2 * _SCALE * bka).astype(np.float32).reshape(1, _NH)
        common["bq16"] = bq.astype(np.float16).reshape(_H, 1)
        # the final bias row carries bt plus the +bq of the mq residual
        common["bt"] = (bt + bq).astype(np.float32).reshape(1, _H)
    return common, with_bias


def kernel(**inputs):
    from concourse import bass_utils

    hs = np.asarray(inputs["hidden_states"], np.float32)
    assert hs.shape == (_B, _S, _H), hs.shape

    common, with_bias = _host_prep(inputs)
    if with_bias not in _BUILT:
        _BUILT[with_bias] = _build(with_bias)
    nc = _BUILT[with_bias]

    in_maps = [dict(common, x=np.ascontiguousarray(hs[b])) for b in range(_B)]
    res = bass_utils.run_bass_kernel_spmd(nc, in_maps, core_ids=list(range(_B)))
    global LAST_RESULTS
    LAST_RESULTS = res
    out = np.stack([r["out"] for r in res.results], axis=0)
    return out.astype(np.float32)


if __name__ == "__main__":
    import sys
    if "--tlsim" in sys.argv:
        from concourse.timeline_sim import TimelineSim
        nc = _build("--bias" in sys.argv)
        tl = TimelineSim(nc, trace=False)
        t = tl.simulate()
        print(f"TimelineSim estimated exec: {t:.0f} ns = {t/1000:.1f} us")
    elif "--sim" in sys.argv:
        from concourse.bass_interp import CoreSim
        sys.path.insert(0, "/root/problem")
        from algebra_check import make_inputs, ref_numpy

        inputs = make_inputs()
        if "--bias" in sys.argv:
            rng = np.random.default_rng(7)
            for k in ("bq", "bk", "bt"):
                inputs[k] = (rng.standard_normal(_H) * 0.02).astype(np.float32)
            for k in ("bqa", "bka"):
                inputs[k] = (rng.standard_normal(_NH) * 0.02).astype(np.float32)
        common, wb = _host_prep(inputs)
        print("with_bias =", wb)
        nc = _build(wb)
        sim = CoreSim(nc)
        for k, v in common.items():
            sim.tensor(k)[:] = v
        sim.tensor("x")[:] = inputs["hidden_states"][0]
        sim.simulate(check_with_hw=False)
        got = np.array(sim.tensor("out"))
        ref = ref_numpy(**inputs)[0]
        err = np.abs(got - ref).max()
        print("sim absmax err:", err, "rel-to-scale:", err / np.abs(ref).max())
